# revision 5
# baseline (speedup 1.0000x reference)
"""Trainium2 Bass kernel for single-head causal attention with projections.

Reference computation (B=4, T=4096, D=1024, H=64):
    qh = q @ Wq; kh = k @ Wk; vh = v @ Wv          # [B,T,H]
    S  = qh @ kh.T / sqrt(H)  (causal masked)       # [B,T,T]
    out = softmax(S) @ vh                           # [B,T,H]

Sharding: 8 cores = 4 batches x 2 query-halves. Each core owns half a
batch's queries (8 tiles of 256 rows, folded pairing so causal work is
balanced) and projects half the kv positions; the pair exchanges the
small projected kh/vh tensors with an in-kernel AllGather (DRAM bounce).

Performance structure (v2 schedule):
  * tensor-engine work is the roofline: ~123k PE cycles/core of real
    matmuls (proj + S + PV).  The scalar engine's exp runs at exactly
    half the PE column rate, so pure-attention stretches are ACT-bound;
    the schedule therefore spreads the projection chains across ALL
    attention blocks so the PE always has surplus work and the HAM clock
    gate stays at full speed with almost no garbage filler.
  * S chunks are packed into 2-bank [128,1024] PSUM tiles (pairs of 512
    or a quad of 256 tails) so one strided ACTIVATE covers the pack,
    amortizing the ~290-cycle ACT instruction overhead.
  * PSUM budget (8 banks): 2 PV accumulators ([65,512]: j0 dedicated,
    j3->j2->j1 sequentially reusing the second bank with interleaved
    drains), 2x2-bank S tiles, 2 projection banks.
  * the v-head fold uses the XBAR dma transpose instead of PE-identity
    transposes (frees PSUM + PE cycles).
  * exp activation table and the CC collective path are pre-warmed with
    dummy ops during the DMA-latency head; a short garbage-matmul primer
    opens the HAM clock gate before real data lands.
  * scalar engine runs ONLY activations; sync issues input streams;
    vector does copies/masks/transposes; gpsimd runs the exchange.
  * output is stored transposed with the softmax denominator as row 65;
    the host does the final divide+transpose.
"""

import numpy as np

B, T, D, H = 4, 4096, 1024, 64
TILE = 256          # tq position tile
NPOS = 8            # q position tiles per core
DC = D // 128       # d chunks
NKV = T // 128      # kv chunks
NG = T // 1024      # kv stream groups (4); each core projects 512 of 1024
TQ = NPOS * TILE    # q rows per core
NPAIR = NPOS // 2
TH = T // 2         # kv columns projected per core

# per-position kv chunk counts (identical across cores): 32,28,...,4
COUNTS = [NKV - 4 * p for p in range(NPOS)]
TILES_H0 = [14 - 2 * p for p in range(NPOS)]
TILES_H1 = [15 - 2 * p for p in range(NPOS)]
REPLICA_GROUPS = [[0, 1], [2, 3], [4, 5], [6, 7]]

_CACHE = {}


def _merge(base_events, attn_events):
    """Interleave two event lists proportionally."""
    nb, na = len(base_events), len(attn_events)
    if na == 0:
        return list(base_events)
    if nb == 0:
        return list(attn_events)
    out = []
    ai = 0
    acc = 0.0
    per = na / nb
    for ev in base_events:
        out.append(ev)
        acc += per
        while acc >= 1.0 and ai < na:
            out.append(attn_events[ai])
            ai += 1
            acc -= 1.0
    out.extend(attn_events[ai:])
    return out


def _zip2(a, b):
    """Strictly alternate two event lists, then leftovers."""
    out = []
    for x, y in zip(a, b):
        out.append(x)
        out.append(y)
    longer = a if len(a) > len(b) else b
    out.extend(longer[min(len(a), len(b)):])
    return out


def _build_program(counts, use_mask):
    import concourse.bacc as bacc
    import concourse.mybir as mybir
    import concourse.tile as tile

    f32 = mybir.dt.float32
    bf16 = mybir.dt.bfloat16

    nc = bacc.Bacc(None, target_bir_lowering=False, debug=False,
                   num_devices=8)
    qT = nc.declare_dram_parameter("qT", [D, TQ], bf16, isOutput=False)
    kT = nc.declare_dram_parameter("kT", [D, TH], bf16, isOutput=False)
    vT = nc.declare_dram_parameter("vT", [D, TH], bf16, isOutput=False)
    wq = nc.declare_dram_parameter("wq", [128, DC, H], bf16,
                                   isOutput=False)
    wk = nc.declare_dram_parameter("wk", [128, DC, H], bf16,
                                   isOutput=False)
    wv = nc.declare_dram_parameter("wv", [128, DC, H], bf16,
                                   isOutput=False)
    pat = nc.declare_dram_parameter("pat", [128, 4, 2 * TILE], bf16,
                                    isOutput=False)
    outT = nc.declare_dram_parameter("outT", [H + 1, TQ], f32,
                                     isOutput=True)

    # [p, c, t] view so one DMA grabs two contiguous d-chunks
    qT2 = qT.rearrange("(c p) t -> p c t", p=128)
    kT2 = kT.rearrange("(c p) t -> p c t", p=128)
    vT2 = vT.rearrange("(c p) t -> p c t", p=128)
    scale = 1.0 / float(np.sqrt(H))

    def width(j, m):
        return 2 * TILE if m < counts[2 * j + 1] else TILE

    with tile.TileContext(nc) as tc:
        with (
            tc.tile_pool(name="singles", bufs=1) as singles,
            tc.tile_pool(name="stream", bufs=12) as stream,
            tc.tile_pool(name="psb", bufs=4) as psbp,
            tc.tile_pool(name="dram", bufs=2, space="DRAM") as dram,
            tc.tile_pool(name="proj_ps", bufs=2, space="PSUM") as pps,
            tc.tile_pool(name="s_ps", bufs=2, space="PSUM") as sps,
            tc.tile_pool(name="pv_ps", bufs=1, space="PSUM") as pvp,
        ):
            wq_sb = singles.tile([128, DC, H], bf16, tag="wq")
            wk_sb = singles.tile([128, DC, H], bf16, tag="wk")
            wv_sb = singles.tile([128, DC, H], bf16, tag="wv")
            nc.sync.dma_start(out=wq_sb, in_=wq[:, :, :])
            nc.sync.dma_start(out=wk_sb, in_=wk[:, :, :])
            nc.sync.dma_start(out=wv_sb, in_=wv[:, :, :])

            qhT = singles.tile([64, TQ], bf16, tag="qhT")
            khT = singles.tile([64, T], bf16, tag="khT")
            vh1 = singles.tile([128, NKV, H + 1], bf16, tag="vh1")

            pat_sb = singles.tile([128, 4, 2 * TILE], bf16, tag="pat")
            nc.sync.dma_start(out=pat_sb, in_=pat[:, :, :])
            nc.vector.memset(vh1[:, :, H:H + 1], 1.0)

            # ---- warm the CC collective path with a dummy AllGather so
            # the first real exchange doesn't pay ring-setup latency and
            # paired cores synchronize their start skew early.
            dki = dram.tile([64, 16], bf16, tag="dki", name="dki")
            dko = dram.tile([128, 16], bf16, tag="dko", name="dko")
            nc.gpsimd.collective_compute(
                "AllGather", mybir.AluOpType.bypass,
                replica_groups=REPLICA_GROUPS,
                ins=[dki.opt()], outs=[dko.opt()])

            # ---- HAM warmup: prime the PE clock gate with back-to-back
            # garbage matmuls while the DMA head is in flight; also fire
            # one tiny Exp to pull the ACT spline tables in early.
            wrm = singles.tile([128, 512], bf16, tag="wrm")
            nc.gpsimd.memset(wrm, 0.5)
            warm_act = singles.tile([128, 16], bf16, tag="warm_act")
            for w in range(20):
                wps = sps.tile([128, 1024], f32, tag="sp", name=f"warm{w}")
                nc.tensor.matmul(wps[:, 0:512], wrm[:, :128], wrm,
                                 start=True, stop=True)
                if w == 0:
                    nc.scalar.activation(
                        warm_act, wps[:, 0:16],
                        mybir.ActivationFunctionType.Exp, scale=0.001)

            fill_n = [0]

            def filler():
                def go():
                    fill_n[0] += 1
                    f = sps.tile([128, 1024], f32, tag="sp",
                                 name=f"fill{fill_n[0]}")
                    nc.tensor.matmul(f[:, 0:512], wrm[:, :128], wrm,
                                     start=True, stop=True)
                return go

            # ---- projection chains -----------------------------------
            def qchain(qg, half):
                ph = pps.tile([64, 512], f32, tag="ph", name=f"phq{qg}{half}")
                col0 = qg * 1024 + half * 512
                evs = []

                def step(c):
                    def go():
                        t = stream.tile([128, 2, 512], bf16, tag="qkv")
                        nc.sync.dma_start(
                            out=t, in_=qT2[:, c:c + 2, col0:col0 + 512])
                        nc.tensor.matmul(ph, wq_sb[:, c, :], t[:, 0, :],
                                         start=(c == 0), stop=False)
                        nc.tensor.matmul(ph, wq_sb[:, c + 1, :], t[:, 1, :],
                                         start=False, stop=(c + 2 == DC))
                    return go
                evs.extend(step(c) for c in range(0, DC, 2))

                def evict():
                    nc.vector.tensor_copy(qhT[:, col0:col0 + 512], ph)
                evs.append(evict)
                return evs

            def kvchain(gg, which, split_first=False):
                """k or v projection for group gg; ends with a local
                fold into khl / vst (no exchange -- see exchange(gg))."""
                src2 = kT2 if which == "k" else vT2
                w_sb = wk_sb if which == "k" else wv_sb
                ph = pps.tile([64, 512], f32, tag="ph",
                              name=f"ph{which}{gg}")
                col0 = gg * 512
                evs = []

                def step(c):
                    def go():
                        t = stream.tile([128, 2, 512], bf16, tag="qkv")
                        if split_first and c == 0:
                            nc.sync.dma_start(
                                out=t[:, 0, :],
                                in_=src2[:, 0, col0:col0 + 512])
                            nc.sync.dma_start(
                                out=t[:, 1, :],
                                in_=src2[:, 1, col0:col0 + 512])
                        else:
                            nc.sync.dma_start(
                                out=t, in_=src2[:, c:c + 2, col0:col0 + 512])
                        nc.tensor.matmul(ph, w_sb[:, c, :], t[:, 0, :],
                                         start=(c == 0), stop=False)
                        nc.tensor.matmul(ph, w_sb[:, c + 1, :], t[:, 1, :],
                                         start=False, stop=(c + 2 == DC))
                    return go
                evs.extend(step(c) for c in range(0, DC, 2))

                hold = {}

                def fold():
                    if which == "k":
                        khl = stream.tile([64, 512], bf16, tag="khl",
                                          bufs=2)
                        nc.vector.tensor_copy(khl, ph)
                        hold["t"] = khl
                    else:
                        vtmp = stream.tile([64, 512], bf16, tag="vtmp",
                                           bufs=2)
                        nc.vector.tensor_copy(vtmp, ph)
                        # XBAR transpose: [64h, 512pos] -> [128, 4, 64]
                        vst = stream.tile([128, 4, H], bf16, tag="vst",
                                          bufs=2)
                        nc.sync.dma_start_transpose(vst, vtmp)
                        hold["t"] = vst
                evs.append(fold)
                return evs, hold

            def exchange(gg, khold, vhold):
                """One AllGather carrying both kh and vh for group gg:
                ebi rows 0-63 cols 0-511 = khl, all rows cols 512-767 =
                vst (flattened)."""
                def go():
                    ebi = dram.tile([128, 768], bf16, tag="ebi",
                                    name=f"ebi{gg}")
                    nc.gpsimd.dma_start(out=ebi[0:64, 0:512],
                                        in_=khold["t"])
                    vst_f = vhold["t"].rearrange("p s h -> p (s h)")
                    nc.gpsimd.dma_start(out=ebi[:, 512:768], in_=vst_f)
                    ebo = dram.tile([256, 768], bf16, tag="ebo",
                                    name=f"ebo{gg}")
                    nc.gpsimd.collective_compute(
                        "AllGather", mybir.AluOpType.bypass,
                        replica_groups=REPLICA_GROUPS,
                        ins=[ebi.opt()], outs=[ebo.opt()])
                    g0 = gg * 1024
                    nc.gpsimd.dma_start(out=khT[:, g0:g0 + 512],
                                        in_=ebo[0:64, 0:512])
                    nc.gpsimd.dma_start(out=khT[:, g0 + 512:g0 + 1024],
                                        in_=ebo[128:192, 0:512])
                    vbo_r = ebo[:, 512:768].rearrange(
                        "(g p) (s h) -> g p s h", p=128, s=4)
                    for half in range(2):
                        nc.gpsimd.dma_start(
                            out=vh1[:, 8 * gg + 4 * half:
                                    8 * gg + 4 * half + 4, 0:H],
                            in_=vbo_r[half, :, :, :])
                return go

            def grp(gg, qc, split_first=False, q_first=False,
                    fillers=0):
                """k+v chains for group gg (interleaved), one combined
                exchange, and q chain `qc`."""
                kevs, khold = kvchain(gg, "k", split_first=split_first)
                vevs, vhold = kvchain(gg, "v")
                kv = _zip2(kevs, vevs) + [exchange(gg, khold, vhold)]
                if fillers:
                    kv = _merge(kv, [filler() for _ in range(fillers)])
                qevs = qchain(*qc)
                return qevs + kv if q_first else kv + qevs

            # ---- attention -------------------------------------------
            # pv accumulators: two PSUM banks, sequentially reused by
            # the j-pairs per pv_tag (drains interleave the handoffs).
            pv_tag = {0: "pvA", 1: "pvB", 2: "pvB", 3: "pvB"} \
                if use_mask else {0: "pvA", 1: "pvB", 2: "pvA", 3: "pvB"}
            pv_tiles = {}

            def pv_tile(j):
                if j not in pv_tiles:
                    pv_tiles[j] = pvp.tile([H + 1, 2 * TILE], f32,
                                           tag=pv_tag[j], name=f"pvt{j}")
                return pv_tiles[j]

            # a "pack" is a list of chunks [(j, m, off, w)] sharing one
            # [128, 1024] S tile and one activation.
            def make_packs(j, m_lo, m_hi):
                packs = []
                cur = []
                off = 0
                for m in range(m_lo, m_hi):
                    if m >= counts[2 * j]:
                        break
                    w = width(j, m)
                    if off + w > 1024 or (w == 512 and off % 512 != 0):
                        packs.append(cur)
                        cur = []
                        off = 0
                    cur.append((j, m, off, w))
                    off += w
                if cur:
                    packs.append(cur)
                return packs

            def s_pack(pack, box):
                def go():
                    sp = sps.tile([128, 1024], f32, tag="sp",
                                  name=f"s{pack[0][0]}_{pack[0][1]}")
                    for (j, m, off, w) in pack:
                        nc.tensor.matmul(
                            sp[:, off:off + w],
                            khT[:, m * 128:(m + 1) * 128],
                            qhT[:, 2 * j * TILE:2 * j * TILE + w],
                            start=True, stop=True)
                    span = pack[-1][2] + pack[-1][3]
                    p = psbp.tile([128, 1024], bf16, tag="p")
                    nc.scalar.activation(
                        p[:, :span], sp[:, :span],
                        mybir.ActivationFunctionType.Exp, scale=scale)
                    if use_mask:
                        for (j, m, off, w) in pack:
                            cL, cR = counts[2 * j], counts[2 * j + 1]
                            if m >= cL - 4:
                                nc.vector.tensor_mul(
                                    p[:, off:off + TILE],
                                    p[:, off:off + TILE],
                                    pat_sb[:, m - cL + 4, :TILE])
                            if w == 2 * TILE and m >= cR - 4:
                                nc.vector.tensor_mul(
                                    p[:, off + TILE:off + 2 * TILE],
                                    p[:, off + TILE:off + 2 * TILE],
                                    pat_sb[:, m - cR + 4, :TILE])
                    box.append(p)
                return go

            def pv_pack(pack, box):
                def go():
                    p = box[0]
                    for (j, m, off, w) in pack:
                        nc.tensor.matmul(
                            pv_tile(j)[:, :w], vh1[:, m, :],
                            p[:, off:off + w],
                            start=(m == 0), stop=(m == counts[2 * j] - 1),
                            skip_group_check=True)
                return go

            def att_events(jobs, lag=1):
                """jobs: list of (j, m_lo, m_hi) runs -> S/PV events with
                a software-pipeline lag (in packs)."""
                packs = []
                for (j, lo, hi) in jobs:
                    packs.extend(make_packs(j, lo, hi))
                evs = []
                pend = []
                for pk in packs:
                    box = []
                    evs.append(s_pack(pk, box))
                    pend.append(pv_pack(pk, box))
                    if len(pend) > lag:
                        evs.append(pend.pop(0))
                evs.extend(pend)
                return evs

            def drain(j):
                def go():
                    dsb = stream.tile([H + 1, 2 * TILE], f32, tag="dsb",
                                      bufs=2)
                    nc.vector.tensor_copy(dsb, pv_tile(j))
                    del pv_tiles[j]
                    for ps, pe in ((0, 33), (33, 65)):
                        nc.sync.dma_start(
                            out=outT[ps:pe, j * 512:(j + 1) * 512],
                            in_=dsb[ps:pe, :])
                return [go]

            if use_mask:
                # blocks: chains emitted with attention merged in; the
                # hard deps (exchange/qchain readiness) are enforced by
                # tile semaphores, emission order just keeps the PE fed.
                c_b0 = grp(0, (0, 0), split_first=True, fillers=6)
                c_b0b = grp(1, (1, 1))
                c_b1 = grp(2, (1, 0), q_first=True)
                c_b2 = grp(3, (0, 1), q_first=True)
                a_b0 = att_events([(0, 0, 8), (3, 0, 8)])
                a_b1 = drain(3) + att_events([(0, 8, 16), (2, 0, 16)])
                a_b2 = drain(2) + att_events([(0, 16, 24), (1, 0, 16)])
                a_b3 = att_events([(1, 16, 24), (0, 24, 32)], lag=2)

                # emission: groups 0/1 run bare (exchange-latency head);
                # attention then merges into the remaining chains.
                for ev in c_b0 + c_b0b:
                    ev()
                for ev in _merge(c_b1 + c_b2, a_b0 + a_b1 + a_b2):
                    ev()
                for ev in a_b3:
                    ev()
                for ev in drain(1) + drain(0):
                    ev()
            else:
                # no causal mask: every pair attends all 32 kv chunks.
                # pvA/pvB each serve two pairs sequentially.
                c_all = (grp(0, (0, 0), split_first=True, fillers=6)
                         + grp(1, (0, 1))
                         + grp(2, (1, 0), q_first=True)
                         + grp(3, (1, 1), q_first=True))
                a01 = att_events([(0, 0, 32), (1, 0, 32)])
                for ev in c_all[:24]:
                    ev()
                for ev in _merge(c_all[24:], a01):
                    ev()
                for ev in drain(0) + drain(1):
                    ev()
                a23 = att_events([(2, 0, 32), (3, 0, 32)], lag=2)
                for ev in a23:
                    ev()
                for ev in drain(2) + drain(3):
                    ev()
    nc.compile()
    return nc


def _get_program(key, counts, use_mask):
    if key not in _CACHE:
        _CACHE[key] = _build_program(counts, use_mask)
    return _CACHE[key]


def _numpy_fallback(q, k, v, mask, Wq, Wk, Wv):
    qh = q.astype(np.float32) @ Wq
    kh = k.astype(np.float32) @ Wk
    vh = v.astype(np.float32) @ Wv
    out = np.empty((B, T, H), np.float32)
    neg = np.float32(-1e30)
    for b in range(B):
        s = (qh[b] @ kh[b].T) / np.float32(np.sqrt(H))
        s = np.where(mask == 0, neg, s)
        s = s - s.max(axis=-1, keepdims=True)
        e = np.exp(s)
        w = e / e.sum(axis=-1, keepdims=True)
        out[b] = w @ vh[b]
    return out


def _w_layout(w, np_in):
    """[D, H] -> [128, DC, H]: partition-major layout for dense DMA."""
    return np.ascontiguousarray(
        w.reshape(DC, 128, H).transpose(1, 0, 2), np_in)


def _make_pat(half):
    """[128, 4, 512] tail-mask patterns; only the first 256 cols are used."""
    tk = np.arange(128)[:, None]
    c = np.arange(2 * TILE)[None, :]
    stair0 = (c >= tk).astype(np.float32)
    stair1 = (c >= 128 + tk).astype(np.float32)
    ones = np.ones((128, 2 * TILE), np.float32)
    zeros = np.zeros((128, 2 * TILE), np.float32)
    if half == 0:
        pats = [stair0, stair1, zeros, zeros]
    else:
        pats = [ones, ones, stair0, stair1]
    return np.stack(pats, axis=1)  # [128, 4, 512]


def _make_in_maps(q, k, v, mask, Wq, Wk, Wv, counts, apply_tail, np_in):
    # kv half-columns owned by core half h: [gg*1024 + h*512, +512) per gg
    half_idx = [
        np.concatenate([np.arange(gg * 1024 + h * 512,
                                  gg * 1024 + h * 512 + 512)
                        for gg in range(NG)])
        for h in range(2)
    ]
    in_maps = []
    metas = []
    for core in range(8):
        b, h = divmod(core, 2)
        tiles = TILES_H0 if h == 0 else TILES_H1
        qT_slab = np.concatenate(
            [q[b, i * TILE:(i + 1) * TILE, :].T for i in tiles], axis=1)
        pat = _make_pat(h) if apply_tail else np.ones(
            (128, 4, 2 * TILE), np.float32)
        kTb = k[b].T
        vTb = v[b].T
        im = {
            "qT": np.ascontiguousarray(qT_slab, np_in),
            "kT": np.ascontiguousarray(kTb[:, half_idx[h]], np_in),
            "vT": np.ascontiguousarray(vTb[:, half_idx[h]], np_in),
            "wq": _w_layout(Wq, np_in), "wk": _w_layout(Wk, np_in),
            "wv": _w_layout(Wv, np_in),
            "pat": np.ascontiguousarray(pat, np_in),
        }
        in_maps.append(im)
        metas.append((b, tiles))
    return in_maps, metas


def kernel(q, k, v, mask, Wq, Wk, Wv):
    from concourse.bass_utils import run_bass_kernel_spmd
    import ml_dtypes

    q = np.ascontiguousarray(q, np.float32)
    k = np.ascontiguousarray(k, np.float32)
    v = np.ascontiguousarray(v, np.float32)
    Wq = np.ascontiguousarray(Wq, np.float32)
    Wk = np.ascontiguousarray(Wk, np.float32)
    Wv = np.ascontiguousarray(Wv, np.float32)
    mask = np.asarray(mask)

    is_tril = bool((mask == np.tril(np.ones((T, T), mask.dtype))).all())
    is_ones = bool((mask == 1).all())
    if not (is_tril or is_ones):
        return _numpy_fallback(q, k, v, mask, Wq, Wk, Wv)

    np_in = ml_dtypes.bfloat16
    counts = COUNTS if is_tril else [NKV] * NPOS
    nc = _get_program(("v7", is_tril), counts, is_tril)

    in_maps, metas = _make_in_maps(
        q, k, v, mask, Wq, Wk, Wv, counts, is_tril, np_in)
    res = run_bass_kernel_spmd(nc, in_maps, list(range(8)))

    out = np.empty((B, T, H), np.float32)
    for c in range(8):
        b, tiles = metas[c]
        oc = res.results[c]["outT"]  # [H+1, TQ]: rows 0..H-1 num, row H den
        slab = (oc[:H, :] / oc[H:H + 1, :]).T  # [TQ, H]
        for p, i in enumerate(tiles):
            out[b, i * TILE:(i + 1) * TILE, :] = \
                slab[p * TILE:(p + 1) * TILE, :]
    return out


# revision 7
# speedup vs baseline: 1.3978x; 1.3978x over previous
"""Trainium2 Bass kernel for single-head causal attention with projections.

Reference computation (B=4, T=4096, D=1024, H=64):
    qh = q @ Wq; kh = k @ Wk; vh = v @ Wv          # [B,T,H]
    S  = qh @ kh.T / sqrt(H)  (causal masked)       # [B,T,T]
    out = softmax(S) @ vh                           # [B,T,H]

Sharding: 8 cores = 4 batches x 2 kv-halves (partial softmax).  Each
core projects the FULL query sequence but only half the kv positions
(interleaved in 128-position blocks: core h owns global kv chunk
2m + h), runs attention of all queries against its own kv half, and
emits a partial numerator [64, T] plus partial denominator [1, T].
The host adds the two partials of a batch and divides -- exact softmax,
no max subtraction needed, and crucially NO device collectives: the
in-kernel AllGather path costs ~21us of CC-engine init plus 4-14us per
op, which would gate attention until ~50us.

Performance structure:
  * tensor-engine work is the roofline (~139k PE cycles/core: 16
    projection chains + S/PV over the 36864-col causal area).  The
    scalar engine's exp runs at exactly half the PE column rate, so
    pure-attention stretches are ACT-bound; the schedule therefore
    spreads projection chains between the attention pairs so the PE
    always has surplus ready work and the HAM clock gate stays open.
  * S chunks are packed in pairs into 2-bank [128,1024] PSUM tiles so
    one ACTIVATE covers both, amortizing the ~290-cycle ACT overhead.
  * 128-block kv interleaving makes the chunk schedule identical on
    both cores of a batch (SPMD) with zero waste, and the causal
    boundary needs just ONE masked pack per query pair, with the
    stair patterns baked per-core into a [128,1024] table.
  * PSUM budget (8 banks): 2 rotating PV accumulators ([65,512], one
    per in-flight query pair), 2x2-bank S tiles, 2 projection banks.
  * the v-head fold uses the XBAR dma transpose straight into the
    [kv, head] layout (no PE-identity transposes, no PSUM traffic).
  * exp activation table is pre-warmed and a short garbage-matmul
    primer opens the HAM clock gate during the DMA-latency head.
  * scalar engine runs ONLY activations; sync issues all DMA;
    vector does copies and the mask multiplies.
  * output is stored transposed with the denominator as row 65; the
    host does the final combine+divide+transpose.
"""

import numpy as np

B, T, D, H = 4, 4096, 1024, 64
DC = D // 128       # d chunks
NKVC = 16           # kv chunks per core (128 positions each)
NPAIR = 8           # query pairs of 512 columns
TQO = T             # q columns per core (full sequence)

_CACHE = {}


def _merge(base_events, attn_events):
    """Interleave two event lists proportionally."""
    nb, na = len(base_events), len(attn_events)
    if na == 0:
        return list(base_events)
    if nb == 0:
        return list(attn_events)
    out = []
    ai = 0
    acc = 0.0
    per = na / nb
    for ev in base_events:
        out.append(ev)
        acc += per
        while acc >= 1.0 and ai < na:
            out.append(attn_events[ai])
            ai += 1
            acc -= 1.0
    out.extend(attn_events[ai:])
    return out


def _zip2(a, b):
    """Strictly alternate two event lists, then leftovers."""
    out = []
    for x, y in zip(a, b):
        out.append(x)
        out.append(y)
    longer = a if len(a) > len(b) else b
    out.extend(longer[min(len(a), len(b)):])
    return out


def _build_program(use_mask):
    import concourse.bacc as bacc
    import concourse.mybir as mybir
    import concourse.tile as tile

    f32 = mybir.dt.float32
    bf16 = mybir.dt.bfloat16

    nc = bacc.Bacc(None, target_bir_lowering=False, debug=False,
                   num_devices=8)
    qT = nc.declare_dram_parameter("qT", [D, TQO], bf16, isOutput=False)
    kT = nc.declare_dram_parameter("kT", [D, T // 2], bf16, isOutput=False)
    vT = nc.declare_dram_parameter("vT", [D, T // 2], bf16, isOutput=False)
    wq = nc.declare_dram_parameter("wq", [128, DC, H], bf16, isOutput=False)
    wk = nc.declare_dram_parameter("wk", [128, DC, H], bf16, isOutput=False)
    wv = nc.declare_dram_parameter("wv", [128, DC, H], bf16, isOutput=False)
    pat = nc.declare_dram_parameter("pat", [128, 1024], bf16, isOutput=False)
    outT = nc.declare_dram_parameter("outT", [H + 1, TQO], f32,
                                     isOutput=True)

    # [p, c, t] view so one DMA grabs two contiguous d-chunks
    qT2 = qT.rearrange("(c p) t -> p c t", p=128)
    kT2 = kT.rearrange("(c p) t -> p c t", p=128)
    vT2 = vT.rearrange("(c p) t -> p c t", p=128)
    scale = 1.0 / float(np.sqrt(H))

    # chunks attended by query pair jj (512 cols at 512*jj)
    def n_chunks(jj):
        return 2 * jj + 2 if use_mask else NKVC

    with tile.TileContext(nc) as tc:
        with (
            tc.tile_pool(name="singles", bufs=1) as singles,
            tc.tile_pool(name="stream", bufs=12) as stream,
            tc.tile_pool(name="psb", bufs=4) as psbp,
            tc.tile_pool(name="proj_ps", bufs=2, space="PSUM") as pps,
            tc.tile_pool(name="s_ps", bufs=2, space="PSUM") as sps,
            tc.tile_pool(name="pv_ps", bufs=2, space="PSUM") as pvp,
        ):
            wq_sb = singles.tile([128, DC, H], bf16, tag="wq")
            wk_sb = singles.tile([128, DC, H], bf16, tag="wk")
            wv_sb = singles.tile([128, DC, H], bf16, tag="wv")
            nc.sync.dma_start(out=wq_sb, in_=wq[:, :, :])
            nc.sync.dma_start(out=wk_sb, in_=wk[:, :, :])
            nc.sync.dma_start(out=wv_sb, in_=wv[:, :, :])

            qhT = singles.tile([64, TQO], bf16, tag="qhT")
            khT = singles.tile([64, T // 2], bf16, tag="khT")
            vh1 = singles.tile([128, NKVC, H + 1], bf16, tag="vh1")

            pat_sb = singles.tile([128, 1024], bf16, tag="pat")
            nc.sync.dma_start(out=pat_sb, in_=pat[:, :])
            nc.vector.memset(vh1[:, :, H:H + 1], 1.0)

            # ---- HAM warmup: prime the PE clock gate with back-to-back
            # garbage matmuls while the DMA head is in flight; also fire
            # one tiny Exp to pull the ACT spline tables in early.
            wrm = singles.tile([128, 512], bf16, tag="wrm")
            nc.gpsimd.memset(wrm, 0.5)
            warm_act = singles.tile([128, 16], bf16, tag="warm_act")
            for w in range(20):
                wps = sps.tile([128, 1024], f32, tag="sp", name=f"warm{w}")
                nc.tensor.matmul(wps[:, 0:512], wrm[:, :128], wrm,
                                 start=True, stop=True)
                if w == 0:
                    nc.scalar.activation(
                        warm_act, wps[:, 0:16],
                        mybir.ActivationFunctionType.Exp, scale=0.001)

            fill_n = [0]

            def filler():
                def go():
                    fill_n[0] += 1
                    f = sps.tile([128, 1024], f32, tag="sp",
                                 name=f"fill{fill_n[0]}")
                    nc.tensor.matmul(f[:, 0:512], wrm[:, :128], wrm,
                                     start=True, stop=True)
                return go

            # ---- projection chains -----------------------------------
            def chain(kind, idx, split_first=False):
                src2 = {"q": qT2, "k": kT2, "v": vT2}[kind]
                w_sb = {"q": wq_sb, "k": wk_sb, "v": wv_sb}[kind]
                ph = pps.tile([64, 512], f32, tag="ph",
                              name=f"ph{kind}{idx}")
                col0 = idx * 512
                evs = []

                def step(c):
                    def go():
                        t = stream.tile([128, 2, 512], bf16, tag="qkv")
                        if split_first and c == 0:
                            nc.sync.dma_start(
                                out=t[:, 0, :],
                                in_=src2[:, 0, col0:col0 + 512])
                            nc.sync.dma_start(
                                out=t[:, 1, :],
                                in_=src2[:, 1, col0:col0 + 512])
                        else:
                            nc.sync.dma_start(
                                out=t, in_=src2[:, c:c + 2, col0:col0 + 512])
                        nc.tensor.matmul(ph, w_sb[:, c, :], t[:, 0, :],
                                         start=(c == 0), stop=False)
                        nc.tensor.matmul(ph, w_sb[:, c + 1, :], t[:, 1, :],
                                         start=False, stop=(c + 2 == DC))
                    return go
                evs.extend(step(c) for c in range(0, DC, 2))

                def evict():
                    if kind == "q":
                        nc.vector.tensor_copy(qhT[:, col0:col0 + 512], ph)
                    elif kind == "k":
                        nc.vector.tensor_copy(khT[:, col0:col0 + 512], ph)
                    else:
                        vtmp = stream.tile([64, 512], bf16, tag="vtmp",
                                           bufs=2)
                        nc.vector.tensor_copy(vtmp, ph)
                        # XBAR transpose [64h, 512pos] -> [128, 4, 64]
                        # (dense dst; the XBAR mangles strided outputs),
                        # then a strided copy into the PV weight layout.
                        vst = stream.tile([128, 4, H], bf16, tag="vst",
                                          bufs=2)
                        nc.sync.dma_start_transpose(vst, vtmp)
                        nc.vector.tensor_copy(
                            vh1[:, 4 * idx:4 * idx + 4, 0:H], vst)
                evs.append(evict)
                return evs

            # ---- attention -------------------------------------------
            def s_pack(jj, m, nm, box):
                """S + exp for chunks (m, m+1) of pair jj; mask the
                pack that straddles the causal diagonal."""
                def go():
                    sp = sps.tile([128, 1024], f32, tag="sp",
                                  name=f"s{jj}_{m}")
                    jc = 512 * jj
                    for i in (0, 1):
                        nc.tensor.matmul(
                            sp[:, 512 * i:512 * i + 512],
                            khT[:, (m + i) * 128:(m + i + 1) * 128],
                            qhT[:, jc:jc + 512],
                            start=True, stop=True)
                    p = psbp.tile([128, 1024], bf16, tag="p")
                    nc.scalar.activation(
                        p, sp, mybir.ActivationFunctionType.Exp,
                        scale=scale)
                    if use_mask and m + 2 == nm:
                        nc.vector.tensor_mul(p, p, pat_sb)
                    box.append(p)
                return go

            def pv_pack(jj, m, nm, pvt, box):
                def go():
                    p = box[0]
                    for i in (0, 1):
                        nc.tensor.matmul(
                            pvt[:, :512], vh1[:, m + i, :],
                            p[:, 512 * i:512 * i + 512],
                            start=(m + i == 0), stop=(m + i == nm - 1),
                            skip_group_check=True)
                return go

            def drain(jj, pvt):
                def go():
                    dsb = stream.tile([H + 1, 512], f32, tag="dsb",
                                      bufs=2)
                    nc.vector.tensor_copy(dsb, pvt)
                    for ps, pe in ((0, 33), (33, 65)):
                        nc.sync.dma_start(
                            out=outT[ps:pe, jj * 512:(jj + 1) * 512],
                            in_=dsb[ps:pe, :])
                return go

            def att_events(pairs, lag=1):
                """Global software pipeline over all pairs: S-pack ...
                PV-pack (lagged), drain(jj) right after a pair's last
                PV."""
                evs = []
                pend = []  # (pv_event, after_event | None)

                def pop():
                    pv, after = pend.pop(0)
                    evs.append(pv)
                    if after is not None:
                        evs.append(after)

                for jj in pairs:
                    nm = n_chunks(jj)
                    pvt = pvp.tile([H + 1, 512], f32, tag="pv",
                                   name=f"pvt{jj}")
                    for m in range(0, nm, 2):
                        box = []
                        evs.append(s_pack(jj, m, nm, box))
                        last = (m + 2 == nm)
                        pend.append((pv_pack(jj, m, nm, pvt, box),
                                     drain(jj, pvt) if last else None))
                        if len(pend) > lag:
                            pop()
                while pend:
                    pop()
                return evs

            if use_mask:
                # head: kv group 0 + q pair 0, with fillers bridging the
                # DMA-latency head; then pairs with the remaining chains
                # merged in so the PE never runs dry.
                head = (_merge(_zip2(chain("k", 0, split_first=True),
                                     chain("v", 0)),
                               [filler() for _ in range(6)])
                        + chain("q", 0))
                segs = [
                    ([0], chain("q", 1)),
                    ([1], chain("k", 1) + chain("v", 1) + chain("q", 2)),
                    ([2], chain("q", 3)),
                    ([3], chain("k", 2) + chain("v", 2) + chain("q", 4)),
                    ([4], chain("q", 5)),
                    ([5], chain("k", 3) + chain("v", 3) + chain("q", 6)),
                    ([6], chain("q", 7)),
                    ([7], []),
                ]
            else:
                head = (_merge(_zip2(chain("k", 0, split_first=True),
                                     chain("v", 0)),
                               [filler() for _ in range(6)])
                        + _zip2(chain("k", 1) + chain("k", 2)
                                + chain("k", 3),
                                chain("v", 1) + chain("v", 2)
                                + chain("v", 3))
                        + chain("q", 0))
                segs = [([jj], chain("q", jj + 1) if jj < 7 else [])
                        for jj in range(NPAIR)]

            for ev in head:
                ev()
            for pairs, chains in segs:
                for ev in _merge(chains, att_events(pairs)):
                    ev()
    nc.compile()
    return nc


def _get_program(key, use_mask):
    if key not in _CACHE:
        _CACHE[key] = _build_program(use_mask)
    return _CACHE[key]


def _numpy_fallback(q, k, v, mask, Wq, Wk, Wv):
    qh = q.astype(np.float32) @ Wq
    kh = k.astype(np.float32) @ Wk
    vh = v.astype(np.float32) @ Wv
    out = np.empty((B, T, H), np.float32)
    neg = np.float32(-1e30)
    for b in range(B):
        s = (qh[b] @ kh[b].T) / np.float32(np.sqrt(H))
        s = np.where(mask == 0, neg, s)
        s = s - s.max(axis=-1, keepdims=True)
        e = np.exp(s)
        w = e / e.sum(axis=-1, keepdims=True)
        out[b] = w @ vh[b]
    return out


def _w_layout(w, np_in):
    """[D, H] -> [128, DC, H]: partition-major layout for dense DMA."""
    return np.ascontiguousarray(
        w.reshape(DC, 128, H).transpose(1, 0, 2), np_in)


def _make_pat(half):
    """[128, 1024] stair mask for the diagonal pack: chunk m=2jj+h*?
    block i covers kv offsets 128*half + 256*i relative to the pair's
    first query column."""
    p = np.arange(128)[:, None]
    c = np.arange(512)[None, :]
    blocks = [(c >= 128 * half + 256 * i + p).astype(np.float32)
              for i in (0, 1)]
    return np.concatenate(blocks, axis=1)  # [128, 1024]


def _make_in_maps(q, k, v, Wq, Wk, Wv, apply_tail, np_in):
    # core h owns global kv chunks 2m + h (128-position blocks)
    idx = [
        np.concatenate([np.arange(128 * (2 * m + h), 128 * (2 * m + h) + 128)
                        for m in range(NKVC)])
        for h in range(2)
    ]
    ones = np.ones((128, 1024), np.float32)
    in_maps = []
    for core in range(8):
        b, h = divmod(core, 2)
        kTb = k[b].T
        vTb = v[b].T
        im = {
            "qT": np.ascontiguousarray(q[b].T, np_in),
            "kT": np.ascontiguousarray(kTb[:, idx[h]], np_in),
            "vT": np.ascontiguousarray(vTb[:, idx[h]], np_in),
            "wq": _w_layout(Wq, np_in), "wk": _w_layout(Wk, np_in),
            "wv": _w_layout(Wv, np_in),
            "pat": np.ascontiguousarray(
                _make_pat(h) if apply_tail else ones, np_in),
        }
        in_maps.append(im)
    return in_maps


def kernel(q, k, v, mask, Wq, Wk, Wv):
    from concourse.bass_utils import run_bass_kernel_spmd
    import ml_dtypes

    q = np.ascontiguousarray(q, np.float32)
    k = np.ascontiguousarray(k, np.float32)
    v = np.ascontiguousarray(v, np.float32)
    Wq = np.ascontiguousarray(Wq, np.float32)
    Wk = np.ascontiguousarray(Wk, np.float32)
    Wv = np.ascontiguousarray(Wv, np.float32)
    mask = np.asarray(mask)

    is_tril = bool((mask == np.tril(np.ones((T, T), mask.dtype))).all())
    is_ones = bool((mask == 1).all())
    if not (is_tril or is_ones):
        return _numpy_fallback(q, k, v, mask, Wq, Wk, Wv)

    np_in = ml_dtypes.bfloat16
    nc = _get_program(("v8", is_tril), is_tril)

    in_maps = _make_in_maps(q, k, v, Wq, Wk, Wv, is_tril, np_in)
    res = run_bass_kernel_spmd(nc, in_maps, list(range(8)))

    out = np.empty((B, T, H), np.float32)
    for b in range(B):
        o0 = res.results[2 * b]["outT"]      # [H+1, T] partials
        o1 = res.results[2 * b + 1]["outT"]
        num = o0[:H, :] + o1[:H, :]
        den = o0[H:H + 1, :] + o1[H:H + 1, :]
        out[b] = (num / den).T
    return out


# revision 9
# speedup vs baseline: 1.4623x; 1.0462x over previous
"""Trainium2 Bass kernel for single-head causal attention with projections.

Reference computation (B=4, T=4096, D=1024, H=64):
    qh = q @ Wq; kh = k @ Wk; vh = v @ Wv          # [B,T,H]
    S  = qh @ kh.T / sqrt(H)  (causal masked)       # [B,T,T]
    out = softmax(S) @ vh                           # [B,T,H]

Sharding: 8 cores = 4 batches x 2 kv-halves (partial softmax).  Each
core projects the FULL query sequence but only half the kv positions
(interleaved in 128-position blocks: core h owns global kv chunk
2m + h), runs attention of all queries against its own kv half, and
emits a partial numerator [64, T] plus partial denominator [1, T].
The host adds the two partials of a batch and divides -- exact softmax,
no max subtraction needed, and crucially NO device collectives: the
in-kernel AllGather path costs ~21us of CC-engine init plus 4-14us per
op, which would gate attention until ~50us.

Performance structure:
  * tensor-engine work is the roofline (~139k PE cycles/core: 16
    projection chains + S/PV over the 36864-col causal area).  The
    scalar engine's exp runs at exactly half the PE column rate, so
    pure-attention stretches are ACT-bound; the schedule therefore
    spreads projection chains between the attention pairs so the PE
    always has surplus ready work and the HAM clock gate stays open.
  * S chunks are packed in pairs into 2-bank [128,1024] PSUM tiles so
    one ACTIVATE covers both, amortizing the ~290-cycle ACT overhead.
  * 128-block kv interleaving makes the chunk schedule identical on
    both cores of a batch (SPMD) with zero waste, and the causal
    boundary needs just ONE masked pack per query pair, with the
    stair patterns baked per-core into a [128,1024] table.
  * PSUM budget (8 banks): 2 rotating PV accumulators ([65,512], one
    per in-flight query pair), 2x2-bank S tiles, 2 projection banks.
  * the v-head fold uses the XBAR dma transpose straight into the
    [kv, head] layout (no PE-identity transposes, no PSUM traffic).
  * exp activation table is pre-warmed and a short garbage-matmul
    primer opens the HAM clock gate during the DMA-latency head.
  * scalar engine runs ONLY activations; sync issues all DMA;
    vector does copies and the mask multiplies.
  * output is stored transposed with the denominator as row 65; the
    host does the final combine+divide+transpose.
"""

import numpy as np

B, T, D, H = 4, 4096, 1024, 64
DC = D // 128       # d chunks
NKVC = 16           # kv chunks per core (128 positions each)
NPAIR = 8           # query pairs of 512 columns
TQO = T             # q columns per core (full sequence)

_CACHE = {}


def _merge(base_events, attn_events):
    """Interleave two event lists proportionally."""
    nb, na = len(base_events), len(attn_events)
    if na == 0:
        return list(base_events)
    if nb == 0:
        return list(attn_events)
    out = []
    ai = 0
    acc = 0.0
    per = na / nb
    for ev in base_events:
        out.append(ev)
        acc += per
        while acc >= 1.0 and ai < na:
            out.append(attn_events[ai])
            ai += 1
            acc -= 1.0
    out.extend(attn_events[ai:])
    return out


def _zip2(a, b):
    """Strictly alternate two event lists, then leftovers."""
    out = []
    for x, y in zip(a, b):
        out.append(x)
        out.append(y)
    longer = a if len(a) > len(b) else b
    out.extend(longer[min(len(a), len(b)):])
    return out


def _build_program(use_mask):
    import concourse.bacc as bacc
    import concourse.mybir as mybir
    import concourse.tile as tile

    f32 = mybir.dt.float32
    bf16 = mybir.dt.bfloat16

    nc = bacc.Bacc(None, target_bir_lowering=False, debug=False,
                   num_devices=8)
    qT = nc.declare_dram_parameter("qT", [D, TQO], bf16, isOutput=False)
    kT = nc.declare_dram_parameter("kT", [D, T // 2], bf16, isOutput=False)
    vT = nc.declare_dram_parameter("vT", [D, T // 2], bf16, isOutput=False)
    wq = nc.declare_dram_parameter("wq", [128, DC, H], bf16, isOutput=False)
    wk = nc.declare_dram_parameter("wk", [128, DC, H], bf16, isOutput=False)
    wv = nc.declare_dram_parameter("wv", [128, DC, H], bf16, isOutput=False)
    pat = nc.declare_dram_parameter("pat", [128, 1024], bf16, isOutput=False)
    outT = nc.declare_dram_parameter("outT", [H + 1, TQO], f32,
                                     isOutput=True)

    # [p, c, t] view so one DMA grabs two contiguous d-chunks
    qT2 = qT.rearrange("(c p) t -> p c t", p=128)
    kT2 = kT.rearrange("(c p) t -> p c t", p=128)
    vT2 = vT.rearrange("(c p) t -> p c t", p=128)
    scale = 1.0 / float(np.sqrt(H))

    # chunks attended by query pair jj (512 cols at 512*jj)
    def n_chunks(jj):
        return 2 * jj + 2 if use_mask else NKVC

    with tile.TileContext(nc) as tc:
        with (
            tc.tile_pool(name="singles", bufs=1) as singles,
            tc.tile_pool(name="stream", bufs=12) as stream,
            tc.tile_pool(name="psb", bufs=4) as psbp,
            tc.tile_pool(name="proj_ps", bufs=2, space="PSUM") as pps,
            tc.tile_pool(name="s_ps", bufs=2, space="PSUM") as sps,
            tc.tile_pool(name="pv_ps", bufs=2, space="PSUM") as pvp,
        ):
            wq_sb = singles.tile([128, DC, H], bf16, tag="wq")
            wk_sb = singles.tile([128, DC, H], bf16, tag="wk")
            wv_sb = singles.tile([128, DC, H], bf16, tag="wv")
            nc.sync.dma_start(out=wq_sb, in_=wq[:, :, :])
            nc.sync.dma_start(out=wk_sb, in_=wk[:, :, :])
            nc.sync.dma_start(out=wv_sb, in_=wv[:, :, :])

            qhT = singles.tile([64, TQO], bf16, tag="qhT")
            khT = singles.tile([64, T // 2], bf16, tag="khT")
            vh1 = singles.tile([128, NKVC, H + 1], bf16, tag="vh1")

            pat_sb = singles.tile([128, 1024], bf16, tag="pat")
            nc.sync.dma_start(out=pat_sb, in_=pat[:, :])
            nc.vector.memset(vh1[:, :, H:H + 1], 1.0)

            # ---- HAM warmup: prime the PE clock gate with back-to-back
            # garbage matmuls while the DMA head is in flight; also fire
            # one tiny Exp to pull the ACT spline tables in early.
            wrm = singles.tile([128, 512], bf16, tag="wrm")
            nc.gpsimd.memset(wrm, 0.5)
            warm_act = singles.tile([128, 16], bf16, tag="warm_act")
            for w in range(20):
                wps = sps.tile([128, 1024], f32, tag="sp", name=f"warm{w}")
                nc.tensor.matmul(wps[:, 0:512], wrm[:, :128], wrm,
                                 start=True, stop=True)
                if w == 0:
                    nc.scalar.activation(
                        warm_act, wps[:, 0:16],
                        mybir.ActivationFunctionType.Exp, scale=0.001)

            fill_n = [0]

            def filler():
                def go():
                    fill_n[0] += 1
                    f = sps.tile([128, 1024], f32, tag="sp",
                                 name=f"fill{fill_n[0]}")
                    nc.tensor.matmul(f[:, 0:512], wrm[:, :128], wrm,
                                     start=True, stop=True)
                return go

            # ---- projection chains -----------------------------------
            # q streams ride the gpsimd DMA queue so input bandwidth
            # comes from two hw queues in parallel; 4-chunk (512KB)
            # tiles keep the issue cost down.
            def chain(kind, idx, split_first=False):
                src2 = {"q": qT2, "k": kT2, "v": vT2}[kind]
                w_sb = {"q": wq_sb, "k": wk_sb, "v": wv_sb}[kind]
                eng = nc.gpsimd if kind == "q" else nc.sync
                ph = pps.tile([64, 512], f32, tag="ph",
                              name=f"ph{kind}{idx}")
                col0 = idx * 512
                evs = []

                def step(c):
                    def go():
                        t = stream.tile([128, 4, 512], bf16, tag="qkv")
                        if split_first and c == 0:
                            for i in range(4):
                                eng.dma_start(
                                    out=t[:, i, :],
                                    in_=src2[:, i, col0:col0 + 512])
                        else:
                            eng.dma_start(
                                out=t, in_=src2[:, c:c + 4, col0:col0 + 512])
                        for i in range(4):
                            nc.tensor.matmul(
                                ph, w_sb[:, c + i, :], t[:, i, :],
                                start=(c + i == 0), stop=(c + i == DC - 1))
                    return go
                evs.extend(step(c) for c in range(0, DC, 4))

                def evict():
                    if kind == "q":
                        nc.vector.tensor_copy(qhT[:, col0:col0 + 512], ph)
                    elif kind == "k":
                        nc.vector.tensor_copy(khT[:, col0:col0 + 512], ph)
                    else:
                        vtmp = stream.tile([64, 512], bf16, tag="vtmp",
                                           bufs=2)
                        nc.vector.tensor_copy(vtmp, ph)
                        # XBAR transpose [64h, 512pos] -> [128, 4, 64]
                        # (dense dst; the XBAR mangles strided outputs),
                        # then a strided copy into the PV weight layout.
                        vst = stream.tile([128, 4, H], bf16, tag="vst",
                                          bufs=2)
                        nc.sync.dma_start_transpose(vst, vtmp)
                        nc.vector.tensor_copy(
                            vh1[:, 4 * idx:4 * idx + 4, 0:H], vst)
                evs.append(evict)
                return evs

            # ---- attention -------------------------------------------
            def s_pack(jj, m, nm, box):
                """S + exp for chunks (m, m+1) of pair jj; mask the
                pack that straddles the causal diagonal."""
                def go():
                    sp = sps.tile([128, 1024], f32, tag="sp",
                                  name=f"s{jj}_{m}")
                    jc = 512 * jj
                    for i in (0, 1):
                        nc.tensor.matmul(
                            sp[:, 512 * i:512 * i + 512],
                            khT[:, (m + i) * 128:(m + i + 1) * 128],
                            qhT[:, jc:jc + 512],
                            start=True, stop=True)
                    p = psbp.tile([128, 1024], bf16, tag="p")
                    nc.scalar.activation(
                        p, sp, mybir.ActivationFunctionType.Exp,
                        scale=scale)
                    if use_mask and m + 2 == nm:
                        nc.vector.tensor_mul(p, p, pat_sb)
                    box.append(p)
                return go

            def pv_pack(jj, m, nm, pvt, box):
                def go():
                    p = box[0]
                    for i in (0, 1):
                        nc.tensor.matmul(
                            pvt[:, :512], vh1[:, m + i, :],
                            p[:, 512 * i:512 * i + 512],
                            start=(m + i == 0), stop=(m + i == nm - 1),
                            skip_group_check=True)
                return go

            def drain(jj, pvt):
                def go():
                    dsb = stream.tile([H + 1, 512], f32, tag="dsb",
                                      bufs=2)
                    nc.vector.tensor_copy(dsb, pvt)
                    for ps, pe in ((0, 33), (33, 65)):
                        nc.sync.dma_start(
                            out=outT[ps:pe, jj * 512:(jj + 1) * 512],
                            in_=dsb[ps:pe, :])
                return go

            def att_events(pairs, lag=1):
                """Global software pipeline over all pairs: S-pack ...
                PV-pack (lagged), drain(jj) right after a pair's last
                PV."""
                evs = []
                pend = []  # (pv_event, after_event | None)

                def pop():
                    pv, after = pend.pop(0)
                    evs.append(pv)
                    if after is not None:
                        evs.append(after)

                for jj in pairs:
                    nm = n_chunks(jj)
                    pvt = pvp.tile([H + 1, 512], f32, tag="pv",
                                   name=f"pvt{jj}")
                    for m in range(0, nm, 2):
                        box = []
                        evs.append(s_pack(jj, m, nm, box))
                        last = (m + 2 == nm)
                        pend.append((pv_pack(jj, m, nm, pvt, box),
                                     drain(jj, pvt) if last else None))
                        if len(pend) > lag:
                            pop()
                while pend:
                    pop()
                return evs

            def with_fill(evs, n):
                return _merge(evs, [filler() for _ in range(n)])

            if use_mask:
                # head: kv group 0 + q pair 0, with fillers bridging the
                # DMA-latency head; then pairs with the remaining chains
                # merged in so the PE never runs dry.  Fillers between
                # chain steps bound every DMA wait to < the ~3.4us HAM
                # window so the clock gate stays open.
                head = (with_fill(_zip2(chain("k", 0, split_first=True),
                                        chain("v", 0)), 6)
                        + with_fill(chain("q", 0), 3))
                segs = [
                    ([0], with_fill(chain("q", 1), 3)),
                    ([1], with_fill(chain("k", 1) + chain("v", 1)
                                    + chain("q", 2), 7)),
                    ([2], with_fill(chain("q", 3), 3)),
                    ([3], with_fill(chain("k", 2) + chain("v", 2)
                                    + chain("q", 4), 7)),
                    ([4], with_fill(chain("q", 5), 2)),
                    ([5], with_fill(chain("k", 3) + chain("v", 3)
                                    + chain("q", 6), 5)),
                    ([6], with_fill(chain("q", 7), 2)),
                    ([7], []),
                ]
            else:
                head = (with_fill(_zip2(chain("k", 0, split_first=True),
                                        chain("v", 0)), 6)
                        + with_fill(_zip2(chain("k", 1) + chain("k", 2)
                                          + chain("k", 3),
                                          chain("v", 1) + chain("v", 2)
                                          + chain("v", 3)), 12)
                        + chain("q", 0))
                segs = [([jj], chain("q", jj + 1) if jj < 7 else [])
                        for jj in range(NPAIR)]

            for ev in head:
                ev()
            for pairs, chains in segs:
                for ev in _merge(chains, att_events(pairs)):
                    ev()
    nc.compile()
    return nc


def _get_program(key, use_mask):
    if key not in _CACHE:
        _CACHE[key] = _build_program(use_mask)
    return _CACHE[key]


def _numpy_fallback(q, k, v, mask, Wq, Wk, Wv):
    qh = q.astype(np.float32) @ Wq
    kh = k.astype(np.float32) @ Wk
    vh = v.astype(np.float32) @ Wv
    out = np.empty((B, T, H), np.float32)
    neg = np.float32(-1e30)
    for b in range(B):
        s = (qh[b] @ kh[b].T) / np.float32(np.sqrt(H))
        s = np.where(mask == 0, neg, s)
        s = s - s.max(axis=-1, keepdims=True)
        e = np.exp(s)
        w = e / e.sum(axis=-1, keepdims=True)
        out[b] = w @ vh[b]
    return out


def _w_layout(w, np_in):
    """[D, H] -> [128, DC, H]: partition-major layout for dense DMA."""
    return np.ascontiguousarray(
        w.reshape(DC, 128, H).transpose(1, 0, 2), np_in)


def _make_pat(half):
    """[128, 1024] stair mask for the diagonal pack: chunk m=2jj+h*?
    block i covers kv offsets 128*half + 256*i relative to the pair's
    first query column."""
    p = np.arange(128)[:, None]
    c = np.arange(512)[None, :]
    blocks = [(c >= 128 * half + 256 * i + p).astype(np.float32)
              for i in (0, 1)]
    return np.concatenate(blocks, axis=1)  # [128, 1024]


def _make_in_maps(q, k, v, Wq, Wk, Wv, apply_tail, np_in):
    # core h owns global kv chunks 2m + h (128-position blocks)
    idx = [
        np.concatenate([np.arange(128 * (2 * m + h), 128 * (2 * m + h) + 128)
                        for m in range(NKVC)])
        for h in range(2)
    ]
    ones = np.ones((128, 1024), np.float32)
    in_maps = []
    for core in range(8):
        b, h = divmod(core, 2)
        kTb = k[b].T
        vTb = v[b].T
        im = {
            "qT": np.ascontiguousarray(q[b].T, np_in),
            "kT": np.ascontiguousarray(kTb[:, idx[h]], np_in),
            "vT": np.ascontiguousarray(vTb[:, idx[h]], np_in),
            "wq": _w_layout(Wq, np_in), "wk": _w_layout(Wk, np_in),
            "wv": _w_layout(Wv, np_in),
            "pat": np.ascontiguousarray(
                _make_pat(h) if apply_tail else ones, np_in),
        }
        in_maps.append(im)
    return in_maps


def kernel(q, k, v, mask, Wq, Wk, Wv):
    from concourse.bass_utils import run_bass_kernel_spmd
    import ml_dtypes

    q = np.ascontiguousarray(q, np.float32)
    k = np.ascontiguousarray(k, np.float32)
    v = np.ascontiguousarray(v, np.float32)
    Wq = np.ascontiguousarray(Wq, np.float32)
    Wk = np.ascontiguousarray(Wk, np.float32)
    Wv = np.ascontiguousarray(Wv, np.float32)
    mask = np.asarray(mask)

    is_tril = bool((mask == np.tril(np.ones((T, T), mask.dtype))).all())
    is_ones = bool((mask == 1).all())
    if not (is_tril or is_ones):
        return _numpy_fallback(q, k, v, mask, Wq, Wk, Wv)

    np_in = ml_dtypes.bfloat16
    nc = _get_program(("v8", is_tril), is_tril)

    in_maps = _make_in_maps(q, k, v, Wq, Wk, Wv, is_tril, np_in)
    res = run_bass_kernel_spmd(nc, in_maps, list(range(8)))

    out = np.empty((B, T, H), np.float32)
    for b in range(B):
        o0 = res.results[2 * b]["outT"]      # [H+1, T] partials
        o1 = res.results[2 * b + 1]["outT"]
        num = o0[:H, :] + o1[:H, :]
        den = o0[H:H + 1, :] + o1[H:H + 1, :]
        out[b] = (num / den).T
    return out


# revision 14
# speedup vs baseline: 1.5600x; 1.0668x over previous
"""Trainium2 Bass kernel for single-head causal attention with projections.

Reference computation (B=4, T=4096, D=1024, H=64):
    qh = q @ Wq; kh = k @ Wk; vh = v @ Wv          # [B,T,H]
    S  = qh @ kh.T / sqrt(H)  (causal masked)       # [B,T,T]
    out = softmax(S) @ vh                           # [B,T,H]

Sharding: 8 cores = 4 batches x 2 kv-halves (partial softmax).  Each
core projects the FULL query sequence but only half the kv positions
(interleaved in 128-position blocks: core h owns global kv chunk
2m + h), runs attention of all queries against its own kv half, and
emits a partial numerator [64, T] plus partial denominator [1, T].
The host adds the two partials of a batch and divides -- exact softmax,
no max subtraction needed, and crucially NO device collectives: the
in-kernel AllGather path costs ~21us of CC-engine init plus 4-14us per
op, which would gate attention until ~50us.

Performance structure:
  * tensor-engine work is the roofline (~139k PE cycles/core: 16
    projection chains + S/PV over the 36864-col causal area).  The
    scalar engine's exp runs at exactly half the PE column rate, so
    pure-attention stretches are ACT-bound; the schedule therefore
    spreads projection chains between the attention pairs so the PE
    always has surplus ready work and the HAM clock gate stays open.
  * S chunks are packed in pairs into 2-bank [128,1024] PSUM tiles so
    one ACTIVATE covers both, amortizing the ~290-cycle ACT overhead.
  * 128-block kv interleaving makes the chunk schedule identical on
    both cores of a batch (SPMD) with zero waste, and the causal
    boundary needs just ONE masked pack per query pair, with the
    stair patterns baked per-core into a [128,1024] table.
  * PSUM budget (8 banks): 2 rotating PV accumulators ([65,512], one
    per in-flight query pair), 2x2-bank S tiles, 2 projection banks.
  * the v-head fold uses the XBAR dma transpose straight into the
    [kv, head] layout (no PE-identity transposes, no PSUM traffic).
  * exp activation table is pre-warmed and a short garbage-matmul
    primer opens the HAM clock gate during the DMA-latency head.
  * scalar engine runs ONLY activations; sync issues all DMA;
    vector does copies and the mask multiplies.
  * output is stored transposed with the denominator as row 65; the
    host does the final combine+divide+transpose.
"""

import numpy as np

B, T, D, H = 4, 4096, 1024, 64
DC = D // 128       # d chunks
NKVC = 16           # kv chunks per core (128 positions each)
NPAIR = 8           # query pairs of 512 columns
TQO = T             # q columns per core (full sequence)

_CACHE = {}


def _merge(base_events, attn_events):
    """Interleave two event lists proportionally."""
    nb, na = len(base_events), len(attn_events)
    if na == 0:
        return list(base_events)
    if nb == 0:
        return list(attn_events)
    out = []
    ai = 0
    acc = 0.0
    per = na / nb
    for ev in base_events:
        out.append(ev)
        acc += per
        while acc >= 1.0 and ai < na:
            out.append(attn_events[ai])
            ai += 1
            acc -= 1.0
    out.extend(attn_events[ai:])
    return out


def _zip2(a, b):
    """Strictly alternate two event lists, then leftovers."""
    out = []
    for x, y in zip(a, b):
        out.append(x)
        out.append(y)
    longer = a if len(a) > len(b) else b
    out.extend(longer[min(len(a), len(b)):])
    return out


def _build_program(use_mask):
    import concourse.bacc as bacc
    import concourse.mybir as mybir
    import concourse.tile as tile

    f32 = mybir.dt.float32
    bf16 = mybir.dt.bfloat16

    nc = bacc.Bacc(None, target_bir_lowering=False, debug=False,
                   num_devices=8)
    # streams are pre-tiled on the host: [chain, 128, DC, 512], fully
    # contiguous per partition so each chain is ONE cheap dma issue.
    qS = nc.declare_dram_parameter("qS", [8, 128, DC, 512], bf16,
                                   isOutput=False)
    kS = nc.declare_dram_parameter("kS", [4, 128, DC, 512], bf16,
                                   isOutput=False)
    vS = nc.declare_dram_parameter("vS", [4, 128, DC, 512], bf16,
                                   isOutput=False)
    wq = nc.declare_dram_parameter("wq", [128, DC, H], bf16, isOutput=False)
    wk = nc.declare_dram_parameter("wk", [128, DC, H], bf16, isOutput=False)
    wv = nc.declare_dram_parameter("wv", [128, DC, H], bf16, isOutput=False)
    pat = nc.declare_dram_parameter("pat", [128, 1024], bf16, isOutput=False)
    outT = nc.declare_dram_parameter("outT", [H + 1, TQO], f32,
                                     isOutput=True)
    scale = 1.0 / float(np.sqrt(H))

    # chunks attended by query pair jj (512 cols at 512*jj)
    def n_chunks(jj):
        return 2 * jj + 2 if use_mask else NKVC

    with tile.TileContext(nc) as tc:
        with (
            tc.tile_pool(name="singles", bufs=1) as singles,
            tc.tile_pool(name="stream", bufs=12) as stream,
            tc.tile_pool(name="psb", bufs=4) as psbp,
            tc.tile_pool(name="proj_ps", bufs=2, space="PSUM") as pps,
            tc.tile_pool(name="s_ps", bufs=2, space="PSUM") as sps,
            tc.tile_pool(name="pv_ps", bufs=2, space="PSUM") as pvp,
        ):
            wq_sb = singles.tile([128, DC, H], bf16, tag="wq")
            wk_sb = singles.tile([128, DC, H], bf16, tag="wk")
            wv_sb = singles.tile([128, DC, H], bf16, tag="wv")
            nc.sync.dma_start(out=wq_sb, in_=wq[:, :, :])
            nc.sync.dma_start(out=wk_sb, in_=wk[:, :, :])
            nc.sync.dma_start(out=wv_sb, in_=wv[:, :, :])

            qhT = singles.tile([64, TQO], bf16, tag="qhT")
            khT = singles.tile([64, T // 2], bf16, tag="khT")
            vh1 = singles.tile([128, NKVC, H + 1], bf16, tag="vh1")

            pat_sb = singles.tile([128, 1024], bf16, tag="pat")
            nc.sync.dma_start(out=pat_sb, in_=pat[:, :])
            nc.vector.memset(vh1[:, :, H:H + 1], 1.0)

            # ---- prefetch the ENTIRE input up front: 16 one-chain DMAs
            # split over the sync and gpsimd hw queues (parallel HBM
            # bandwidth); proj chains then never wait on fine-grained
            # pacing, just on their one tile landing.
            stiles = {}
            for kind, idx, src, eng, split in (
                ("k", 0, kS, nc.sync, True), ("q", 0, qS, nc.gpsimd, True),
                ("v", 0, vS, nc.sync, False), ("q", 1, qS, nc.gpsimd, False),
                ("k", 1, kS, nc.sync, False), ("q", 2, qS, nc.gpsimd, False),
                ("v", 1, vS, nc.sync, False), ("q", 3, qS, nc.gpsimd, False),
                ("k", 2, kS, nc.sync, False), ("q", 5, qS, nc.gpsimd, False),
                ("v", 2, vS, nc.sync, False), ("q", 7, qS, nc.gpsimd, False),
                ("k", 3, kS, nc.sync, False),
                ("v", 3, vS, nc.sync, False),
                ("q", 4, qS, nc.sync, False),
                ("q", 6, qS, nc.sync, False),
            ):
                t = stream.tile([128, DC, 512], bf16, tag="qkv", bufs=16,
                                name=f"st_{kind}{idx}")
                if split:
                    eng.dma_start(out=t[:, 0:4, :], in_=src[idx, :, 0:4, :])
                    eng.dma_start(out=t[:, 4:8, :], in_=src[idx, :, 4:8, :])
                else:
                    eng.dma_start(out=t, in_=src[idx, :, :, :])
                stiles[(kind, idx)] = t

            # ---- HAM warmup: prime the PE clock gate with back-to-back
            # garbage matmuls while the DMA head is in flight; also fire
            # one tiny Exp to pull the ACT spline tables in early.
            wrm = singles.tile([128, 512], bf16, tag="wrm")
            nc.gpsimd.memset(wrm, 0.5)
            warm_act = singles.tile([128, 16], bf16, tag="warm_act")
            for w in range(20):
                wps = sps.tile([128, 1024], f32, tag="sp", name=f"warm{w}")
                nc.tensor.matmul(wps[:, 0:512], wrm[:, :128], wrm,
                                 start=True, stop=True)
                if w == 0:
                    nc.scalar.activation(
                        warm_act, wps[:, 0:16],
                        mybir.ActivationFunctionType.Exp, scale=0.001)

            fill_n = [0]

            def filler():
                def go():
                    fill_n[0] += 1
                    f = sps.tile([128, 1024], f32, tag="sp",
                                 name=f"fill{fill_n[0]}")
                    nc.tensor.matmul(f[:, 0:512], wrm[:, :128], wrm,
                                     start=True, stop=True)
                return go

            # ---- projection chains (pure compute; streams prefetched)
            def chain(kind, idx, split_first=False):
                w_sb = {"q": wq_sb, "k": wk_sb, "v": wv_sb}[kind]
                ph = pps.tile([64, 512], f32, tag="ph",
                              name=f"ph{kind}{idx}")
                col0 = idx * 512
                t = stiles[(kind, idx)]
                evs = []

                def step(c):
                    def go():
                        for i in range(4):
                            nc.tensor.matmul(
                                ph, w_sb[:, c + i, :], t[:, c + i, :],
                                start=(c + i == 0), stop=(c + i == DC - 1))
                    return go
                evs.extend(step(c) for c in range(0, DC, 4))

                def evict():
                    if kind == "q":
                        nc.vector.tensor_copy(qhT[:, col0:col0 + 512], ph)
                    elif kind == "k":
                        nc.vector.tensor_copy(khT[:, col0:col0 + 512], ph)
                    else:
                        vtmp = stream.tile([64, 512], bf16, tag="vtmp",
                                           bufs=2)
                        nc.vector.tensor_copy(vtmp, ph)
                        # XBAR transpose [64h, 512pos] -> [128, 4, 64]
                        # (dense dst; the XBAR mangles strided outputs),
                        # then a strided copy into the PV weight layout.
                        vst = stream.tile([128, 4, H], bf16, tag="vst",
                                          bufs=2)
                        nc.sync.dma_start_transpose(vst, vtmp)
                        nc.vector.tensor_copy(
                            vh1[:, 4 * idx:4 * idx + 4, 0:H], vst)
                evs.append(evict)
                return evs

            # ---- attention -------------------------------------------
            def s_pack(jj, m, nm, box):
                """S + exp for chunks (m, m+1) of pair jj; mask the
                pack that straddles the causal diagonal."""
                def go():
                    sp = sps.tile([128, 1024], f32, tag="sp",
                                  name=f"s{jj}_{m}")
                    jc = 512 * jj
                    for i in (0, 1):
                        nc.tensor.matmul(
                            sp[:, 512 * i:512 * i + 512],
                            khT[:, (m + i) * 128:(m + i + 1) * 128],
                            qhT[:, jc:jc + 512],
                            start=True, stop=True)
                    p = psbp.tile([128, 1024], bf16, tag="p")
                    nc.scalar.activation(
                        p, sp, mybir.ActivationFunctionType.Exp,
                        scale=scale)
                    if use_mask and m + 2 == nm:
                        nc.vector.tensor_mul(p, p, pat_sb)
                    box.append(p)
                return go

            def pv_pack(jj, m, nm, pvt, box):
                def go():
                    p = box[0]
                    for i in (0, 1):
                        nc.tensor.matmul(
                            pvt[:, :512], vh1[:, m + i, :],
                            p[:, 512 * i:512 * i + 512],
                            start=(m + i == 0), stop=(m + i == nm - 1),
                            skip_group_check=True)
                return go

            def drain(jj, pvt):
                def go():
                    dsb = stream.tile([H + 1, 512], f32, tag="dsb",
                                      bufs=2)
                    nc.vector.tensor_copy(dsb, pvt)
                    for ps, pe in ((0, 33), (33, 65)):
                        nc.sync.dma_start(
                            out=outT[ps:pe, jj * 512:(jj + 1) * 512],
                            in_=dsb[ps:pe, :])
                return go

            def att_events(pairs, lag=1):
                """Global software pipeline over all pairs: S-pack ...
                PV-pack (lagged), drain(jj) right after a pair's last
                PV."""
                evs = []
                pend = []  # (pv_event, after_event | None)

                def pop():
                    pv, after = pend.pop(0)
                    evs.append(pv)
                    if after is not None:
                        evs.append(after)

                for jj in pairs:
                    nm = n_chunks(jj)
                    pvt = pvp.tile([H + 1, 512], f32, tag="pv",
                                   name=f"pvt{jj}")
                    for m in range(0, nm, 2):
                        box = []
                        evs.append(s_pack(jj, m, nm, box))
                        last = (m + 2 == nm)
                        pend.append((pv_pack(jj, m, nm, pvt, box),
                                     drain(jj, pvt) if last else None))
                        if len(pend) > lag:
                            pop()
                while pend:
                    pop()
                return evs

            def with_fill(evs, n):
                return _merge(evs, [filler() for _ in range(n)])

            if use_mask:
                # head: kv group 0 + q pair 0, with fillers bridging the
                # DMA-latency head; then pairs with the remaining chains
                # merged in so the PE never runs dry.  Fillers between
                # chain steps bound every DMA wait to < the ~3.4us HAM
                # window so the clock gate stays open.
                head = (with_fill(_zip2(chain("k", 0, split_first=True),
                                        chain("v", 0)), 6)
                        + with_fill(chain("q", 0), 3))
                segs = [
                    ([0], with_fill(chain("q", 1), 3)),
                    ([1], with_fill(chain("k", 1) + chain("v", 1)
                                    + chain("q", 2), 7)),
                    ([2], with_fill(chain("q", 3), 3)),
                    ([3], with_fill(chain("k", 2) + chain("v", 2)
                                    + chain("q", 4), 7)),
                    ([4], with_fill(chain("q", 5), 2)),
                    ([5], with_fill(chain("k", 3) + chain("v", 3)
                                    + chain("q", 6), 5)),
                    ([6], with_fill(chain("q", 7), 2)),
                    ([7], []),
                ]
            else:
                head = (with_fill(_zip2(chain("k", 0, split_first=True),
                                        chain("v", 0)), 6)
                        + with_fill(_zip2(chain("k", 1) + chain("k", 2)
                                          + chain("k", 3),
                                          chain("v", 1) + chain("v", 2)
                                          + chain("v", 3)), 12)
                        + chain("q", 0))
                segs = [([jj], chain("q", jj + 1) if jj < 7 else [])
                        for jj in range(NPAIR)]

            for ev in head:
                ev()
            for pairs, chains in segs:
                for ev in _merge(chains, att_events(pairs)):
                    ev()
    nc.compile()
    return nc


def _get_program(key, use_mask):
    if key not in _CACHE:
        _CACHE[key] = _build_program(use_mask)
    return _CACHE[key]


def _numpy_fallback(q, k, v, mask, Wq, Wk, Wv):
    qh = q.astype(np.float32) @ Wq
    kh = k.astype(np.float32) @ Wk
    vh = v.astype(np.float32) @ Wv
    out = np.empty((B, T, H), np.float32)
    neg = np.float32(-1e30)
    for b in range(B):
        s = (qh[b] @ kh[b].T) / np.float32(np.sqrt(H))
        s = np.where(mask == 0, neg, s)
        s = s - s.max(axis=-1, keepdims=True)
        e = np.exp(s)
        w = e / e.sum(axis=-1, keepdims=True)
        out[b] = w @ vh[b]
    return out


def _w_layout(w, np_in):
    """[D, H] -> [128, DC, H]: partition-major layout for dense DMA."""
    return np.ascontiguousarray(
        w.reshape(DC, 128, H).transpose(1, 0, 2), np_in)


def _make_pat(half):
    """[128, 1024] stair mask for the diagonal pack: chunk m=2jj+h*?
    block i covers kv offsets 128*half + 256*i relative to the pair's
    first query column."""
    p = np.arange(128)[:, None]
    c = np.arange(512)[None, :]
    blocks = [(c >= 128 * half + 256 * i + p).astype(np.float32)
              for i in (0, 1)]
    return np.concatenate(blocks, axis=1)  # [128, 1024]


def _stream_pack(slabT, np_in):
    """[D, ncols] -> [ncols//512, 128, DC, 512] chain-major stream."""
    ncols = slabT.shape[1]
    return np.ascontiguousarray(
        slabT.reshape(DC, 128, ncols // 512, 512).transpose(2, 1, 0, 3),
        np_in)


def _make_in_maps(q, k, v, Wq, Wk, Wv, apply_tail, np_in):
    # core h owns global kv chunks 2m + h (128-position blocks)
    idx = [
        np.concatenate([np.arange(128 * (2 * m + h), 128 * (2 * m + h) + 128)
                        for m in range(NKVC)])
        for h in range(2)
    ]
    ones = np.ones((128, 1024), np.float32)
    in_maps = []
    for core in range(8):
        b, h = divmod(core, 2)
        im = {
            "qS": _stream_pack(q[b].T, np_in),
            "kS": _stream_pack(k[b].T[:, idx[h]], np_in),
            "vS": _stream_pack(v[b].T[:, idx[h]], np_in),
            "wq": _w_layout(Wq, np_in), "wk": _w_layout(Wk, np_in),
            "wv": _w_layout(Wv, np_in),
            "pat": np.ascontiguousarray(
                _make_pat(h) if apply_tail else ones, np_in),
        }
        in_maps.append(im)
    return in_maps


def kernel(q, k, v, mask, Wq, Wk, Wv):
    from concourse.bass_utils import run_bass_kernel_spmd
    import ml_dtypes

    q = np.ascontiguousarray(q, np.float32)
    k = np.ascontiguousarray(k, np.float32)
    v = np.ascontiguousarray(v, np.float32)
    Wq = np.ascontiguousarray(Wq, np.float32)
    Wk = np.ascontiguousarray(Wk, np.float32)
    Wv = np.ascontiguousarray(Wv, np.float32)
    mask = np.asarray(mask)

    is_tril = bool((mask == np.tril(np.ones((T, T), mask.dtype))).all())
    is_ones = bool((mask == 1).all())
    if not (is_tril or is_ones):
        return _numpy_fallback(q, k, v, mask, Wq, Wk, Wv)

    np_in = ml_dtypes.bfloat16
    nc = _get_program(("v9", is_tril), is_tril)

    in_maps = _make_in_maps(q, k, v, Wq, Wk, Wv, is_tril, np_in)
    res = run_bass_kernel_spmd(nc, in_maps, list(range(8)))

    out = np.empty((B, T, H), np.float32)
    for b in range(B):
        o0 = res.results[2 * b]["outT"]      # [H+1, T] partials
        o1 = res.results[2 * b + 1]["outT"]
        num = o0[:H, :] + o1[:H, :]
        den = o0[H:H + 1, :] + o1[H:H + 1, :]
        out[b] = (num / den).T
    return out
